# revision 10
# baseline (speedup 1.0000x reference)
"""vq_codebook Trainium2 kernel: pos-encode + masked k-means + proj MLP.

Sharding: pure data parallel over K=8 objects, one object per NeuronCore.

Per-core algorithm (all fp32 — k-means argmin margins are ~1e-5, bf16/fp22
token or distance compression empirically breaks the labels and the final
output; validated vs the jax reference at ~1e-6 rel err in numpy):

  pass 0:  stream feat tiles [128 tok, 768], build tokens = (feat + pos)*mask
           on DVE/GPSIMD/ACT, write tokens to HBM in BOTH layouts
           (natural [16384,768] and transposed [768,16384] via PE transposes),
           and run k-means iteration 1 fused (tiles already in SBUF).
  iters:   4 more k-means iterations; per tile: G = tokens @ c.T via
           6 stationary-tokensT matmuls -> psum [128,10]; q = G - cn2/2;
           DVE max + one-hot U (is_ge vs row max, masked); cluster sums via
           6 stationary-tokens matmuls accumulating psum [128d, 10c] over all
           tiles; counts via ones-stationary matmul.
  update:  counts broadcast (ones-row matmul), divide, select (empty clusters
           keep old centroid), cn2 via ones-col matmul + reduce.
  MLP:     h1 = gelu(c @ W1 + b1); out = h1 @ W2 + b2, PE transposes for h1.
"""

import numpy as np
from contextlib import ExitStack

import concourse.bass as bass
import concourse.bacc as bacc
import concourse.tile as tile
from concourse import mybir
from concourse.bass_utils import run_bass_kernel_spmd

import os
F32 = mybir.dt.float32
_GELU = (mybir.ActivationFunctionType.Identity
         if os.environ.get("KBDBG_NOGELU") else
         mybir.ActivationFunctionType.Gelu)
OP = mybir.AluOpType
AF = mybir.ActivationFunctionType

K, H, W, D, C, ITERS = 8, 128, 128, 768, 10, 5
NT = H * W            # 16384 tokens
NB = D // 128         # 6 d-blocks
NTILE = NT // 128     # 128 token tiles
RAW_H = RAW_W = 1024

COFF = {}
_off = 0
for _n, _w in [("xb", 768), ("wy", 768), ("ygb", 128), ("mt", 128),
               ("c0t", 60), ("ncn0", 10), ("ident", 128),
               ("b1b", 768), ("b2b", 768), ("w1", 4608), ("w2", 4608)]:
    COFF[_n] = _off
    _off += _w
CW = _off

_CACHE = {}


def _build_program():
    nc = bacc.Bacc("TRN2", target_bir_lowering=False, debug=False, num_devices=K)

    feat = nc.dram_tensor("feat", [NT, D], F32, kind="ExternalInput").ap()
    cst_d = nc.dram_tensor("consts", [128, CW], F32, kind="ExternalInput").ap()
    out_d = nc.dram_tensor("out", [C, D], F32, kind="ExternalOutput").ap()

    tokN = nc.dram_tensor("tokn", [NT, D], F32).ap()
    tokT = nc.dram_tensor("tokt", [D, NT], F32).ap()
    # [d, t] -> [dlow, b, t] view for block DMA
    tokT3 = tokT.rearrange("(b p) t -> p b t", b=NB)

    with tile.TileContext(nc) as tc, ExitStack() as ctx:
        const = ctx.enter_context(tc.tile_pool(name="const", bufs=1))
        io = ctx.enter_context(tc.tile_pool(name="io", bufs=3))
        tmp = ctx.enter_context(tc.tile_pool(name="tmp", bufs=2))
        small = ctx.enter_context(tc.tile_pool(name="small", bufs=4))
        ctp = ctx.enter_context(tc.tile_pool(name="ctp", bufs=2))
        ps_tr = ctx.enter_context(tc.tile_pool(name="ps_tr", bufs=2, space="PSUM"))
        ps_g = ctx.enter_context(tc.tile_pool(name="ps_g", bufs=2, space="PSUM"))
        ps_acc = ctx.enter_context(tc.tile_pool(name="ps_acc", bufs=1, space="PSUM"))
        ps_cnt = ctx.enter_context(tc.tile_pool(name="ps_cnt", bufs=1, space="PSUM"))

        cst = const.tile([128, CW], F32, tag="cst")
        nc.sync.dma_start(cst[:, :], cst_d)
        xb = cst[:, COFF["xb"]:COFF["xb"] + D]
        wy = cst[:, COFF["wy"]:COFF["wy"] + D]
        ygb = cst[:, COFF["ygb"]:COFF["ygb"] + H]
        mT = cst[:, COFF["mt"]:COFF["mt"] + H]
        c0t = cst[:, COFF["c0t"]:COFF["c0t"] + NB * C]
        ncn0 = cst[:, COFF["ncn0"]:COFF["ncn0"] + C]
        ident = cst[:, COFF["ident"]:COFF["ident"] + 128]
        b1b = cst[0:C, COFF["b1b"]:COFF["b1b"] + D]
        b2b = cst[0:C, COFF["b2b"]:COFF["b2b"] + D]
        w1 = cst[:, COFF["w1"]:COFF["w1"] + NB * D]
        w2 = cst[:, COFF["w2"]:COFF["w2"] + NB * D]
        ones_c = const.tile([128, 1], F32, tag="ones_c")
        nc.gpsimd.memset(ones_c[:, :], 1.0)
        ones_r = const.tile([1, 128], F32, tag="ones_r")
        nc.gpsimd.memset(ones_r[:, :], 1.0)

        def iter_tile_body(t_i, tokT_sb, tokN_sb, cT, ncnb, psS, psCnt, first):
            """One 128-token tile of one k-means iteration.

            tokT_sb: [128, NB*128] transposed blocks; tokN_sb: [128, D] natural.
            cT: [128, NB*C] centroids (d-blocks x c); ncnb: [128, C] = -cn2/2.
            """
            psG = ps_g.tile([128, 16], F32, tag="g")
            for b in range(NB):
                nc.tensor.matmul(
                    psG[:, 0:C],
                    tokT_sb[:, b * 128:(b + 1) * 128],
                    cT[:, b * C:(b + 1) * C],
                    start=(b == 0), stop=(b == NB - 1),
                )
            q = small.tile([128, C], F32, tag="q")
            nc.vector.tensor_tensor(q[:, :], psG[:, 0:C], ncnb[:, :], op=OP.add)
            mx = small.tile([128, 8], F32, tag="mx")
            nc.vector.max(mx[:, :], q[:, :])
            u = small.tile([128, C], F32, tag="u")
            nc.vector.tensor_scalar(
                u[:, :], q[:, :], mx[:, 0:1], mT[:, t_i:t_i + 1],
                op0=OP.is_ge, op1=OP.mult,
            )
            for b in range(NB):
                nc.tensor.matmul(
                    psS[:, b * C:(b + 1) * C],
                    tokN_sb[:, b * 128:(b + 1) * 128],
                    u[:, :],
                    start=(first and b == 0),
                    stop=(t_i == NTILE - 1 and b == NB - 1),
                )
            nc.tensor.matmul(
                psCnt[:, 0:C], ones_c[:, :], u[:, :],
                start=first, stop=(t_i == NTILE - 1),
            )
            return u

        def iter_finish(psS, psCnt, cT_prev):
            """counts -> new centroids cT [128, NB*C] and ncnb [128, C]."""
            cnt_r = small.tile([1, C], F32, tag="cntr")
            nc.vector.tensor_copy(cnt_r[:, :], psCnt[0:1, 0:C])
            psB = ps_g.tile([128, 16], F32, tag="g")
            nc.tensor.matmul(psB[:, 0:C], ones_r[:, :], cnt_r[:, :],
                             start=True, stop=True)
            cb = small.tile([128, C], F32, tag="cb")
            nc.vector.tensor_copy(cb[:, :], psB[:, 0:C])
            cmax = small.tile([128, C], F32, tag="cmax")
            nc.vector.tensor_scalar(cmax[:, :], cb[:, :], 1.0, None, op0=OP.max)
            msk = small.tile([128, C], mybir.dt.int32, tag="msk")
            nc.vector.tensor_scalar(msk[:, :], cb[:, :], 0.5, None, op0=OP.is_ge)
            rcp = small.tile([128, C], F32, tag="rcp")
            nc.vector.reciprocal(rcp[:, :], cmax[:, :])
            cnew = ctp.tile([128, NB * C], F32, tag="cnew")
            for b in range(NB):
                sl = slice(b * C, (b + 1) * C)
                nc.vector.tensor_tensor(cnew[:, sl], psS[:, sl], rcp[:, :],
                                        op=OP.mult)
            cfin = ctp.tile([128, NB * C], F32, tag="cfin")
            for b in range(NB):
                sl = slice(b * C, (b + 1) * C)
                nc.vector.select(cfin[:, sl], msk[:, :], cnew[:, sl], cT_prev[:, sl])
            sq = tmp.tile([128, NB * C], F32, tag="sq")
            nc.vector.tensor_tensor(sq[:, :], cfin[:, :], cfin[:, :], op=OP.mult)
            psN = ps_cnt.tile([1, 64], F32, tag="cnt")
            nc.tensor.matmul(psN[0:1, 0:NB * C], ones_c[:, :], sq[:, :],
                             start=True, stop=True)
            nr = small.tile([1, C], F32, tag="nr")
            nc.vector.tensor_reduce(
                nr[:, :], psN[0:1, 0:NB * C].rearrange("p (b j) -> p j b", b=NB),
                axis=mybir.AxisListType.X, op=OP.add,
            )
            nr2 = small.tile([1, C], F32, tag="nr2")
            nc.vector.tensor_scalar(nr2[:, :], nr[:, :], -0.5, None, op0=OP.mult)
            psB2 = ps_g.tile([128, 16], F32, tag="g")
            nc.tensor.matmul(psB2[:, 0:C], ones_r[:, :], nr2[:, :],
                             start=True, stop=True)
            ncnb = ctp.tile([128, C], F32, tag="ncnb")
            nc.vector.tensor_copy(ncnb[:, :], psB2[:, 0:C])
            return cfin, ncnb

        # ---------------- pass 0 + fused k-means iter 1 ----------------
        psS = ps_acc.tile([128, NB * C], F32, tag="acc")
        psCnt = ps_cnt.tile([1, 16], F32, tag="cnt")
        for t_i in range(NTILE):
            ft = io.tile([128, D], F32, tag="ft")
            nc.sync.dma_start(ft[:, :], feat[t_i * 128:(t_i + 1) * 128, :])
            t1 = tmp.tile([128, D], F32, tag="t1")
            nc.vector.scalar_tensor_tensor(
                t1[:, :], wy, ygb[:, t_i:t_i + 1], ft[:, :],
                op0=OP.mult, op1=OP.add,
            )
            t2 = tmp.tile([128, D], F32, tag="t2")
            nc.gpsimd.tensor_tensor(t2[:, :], t1[:, :], xb, op=OP.add)
            tok = io.tile([128, D], F32, tag="tok")
            nc.scalar.mul(tok[:, :], t2[:, :], mT[:, t_i:t_i + 1])
            nc.sync.dma_start(tokN[t_i * 128:(t_i + 1) * 128, :], tok[:, :])
            ptr = ps_tr.tile([128, D], F32, tag="tr")
            tokT_sb = io.tile([128, D], F32, tag="tokt")
            for b in range(NB):
                sl = slice(b * 128, (b + 1) * 128)
                nc.tensor.transpose(ptr[:, sl], tok[:, sl], ident)
                if b % 2 == 0:
                    nc.vector.tensor_copy(tokT_sb[:, sl], ptr[:, sl])
                else:
                    nc.scalar.copy(tokT_sb[:, sl], ptr[:, sl])
            nc.sync.dma_start(
                tokT3[:, :, t_i * 128:(t_i + 1) * 128],
                tokT_sb[:, :].rearrange("p (b t) -> p b t", b=NB),
            )
            iter_tile_body(t_i, tokT_sb, tok, c0t, ncn0, psS, psCnt,
                           first=(t_i == 0))
        cT, ncnb = iter_finish(psS, psCnt, c0t)

        # ---------------- k-means iterations 2..ITERS ----------------
        for it in range(1, ITERS):
            psS = ps_acc.tile([128, NB * C], F32, tag="acc")
            psCnt = ps_cnt.tile([1, 16], F32, tag="cnt")
            for t_i in range(0, NTILE, 2):
                pairN = io.tile([128, 2 * D], F32, tag="tok")
                nc.sync.dma_start(
                    pairN[:, :].rearrange("p (i e) -> p i e", i=2),
                    feat_pair_src(tokN, t_i),
                )
                pairT = io.tile([128, 2 * D], F32, tag="tokt")
                for i in range(2):
                    nc.sync.dma_start(
                        pairT[:, i * D:(i + 1) * D]
                            .rearrange("p (b t) -> p b t", b=NB),
                        tokT3[:, :, (t_i + i) * 128:(t_i + i + 1) * 128],
                    )
                for i in range(2):
                    iter_tile_body(
                        t_i + i,
                        pairT[:, i * D:(i + 1) * D],
                        pairN[:, i * D:(i + 1) * D],
                        cT, ncnb, psS, psCnt,
                        first=(t_i + i == 0),
                    )
            cT, ncnb = iter_finish(psS, psCnt, cT)

        # ---------------- projection MLP ----------------
        h1 = tmp.tile([C, D], F32, tag="h1")
        for half in range(2):
            hs = slice(half * (D // 2), (half + 1) * (D // 2))
            psH = ps_g.tile([C, D // 2], F32, tag="g")
            for b in range(NB):
                nc.tensor.matmul(
                    psH[:, :], cT[:, b * C:(b + 1) * C],
                    w1[:, b * D + half * (D // 2): b * D + (half + 1) * (D // 2)],
                    start=(b == 0), stop=(b == NB - 1),
                )
            hb = tmp.tile([C, D // 2], F32, tag="hb")
            nc.vector.tensor_tensor(hb[:, :], psH[:, :], b1b[:, hs], op=OP.add)
            nc.scalar.activation(h1[:, hs], hb[:, :], _GELU)
        h1t = tmp.tile([128, NB * C], F32, tag="h1t")
        for b in range(NB):
            psT = ps_g.tile([128, 16], F32, tag="g")
            nc.tensor.transpose(psT[:, 0:C], h1[:, b * 128:(b + 1) * 128],
                                ident[0:C, 0:C])
            nc.vector.tensor_copy(h1t[:, b * C:(b + 1) * C], psT[:, 0:C])
        osb = tmp.tile([C, D], F32, tag="osb")
        for half in range(2):
            hs = slice(half * (D // 2), (half + 1) * (D // 2))
            psO = ps_g.tile([C, D // 2], F32, tag="g")
            for b in range(NB):
                nc.tensor.matmul(
                    psO[:, :], h1t[:, b * C:(b + 1) * C],
                    w2[:, b * D + half * (D // 2): b * D + (half + 1) * (D // 2)],
                    start=(b == 0), stop=(b == NB - 1),
                )
            nc.vector.tensor_tensor(osb[:, hs], psO[:, :], b2b[:, hs], op=OP.add)
        nc.sync.dma_start(out_d, osb[:, :])

    nc.compile()
    return nc


def feat_pair_src(tokN, t_i):
    return (tokN[t_i * 128:(t_i + 2) * 128, :]
            .rearrange("(i p) e -> p i e", i=2))


def _host_prep(feat, mask, boxes, Wp, bp, W1, b1, W2, b2, init_idx):
    """Per-core input dicts. All small O(K*(H+W)*D) prep; feat is viewed."""
    feat = np.ascontiguousarray(np.asarray(feat, dtype=np.float32))
    mask = np.asarray(mask, dtype=np.float32)
    boxes = np.asarray(boxes, dtype=np.float32)
    Wp = np.asarray(Wp, dtype=np.float32)
    bp = np.asarray(bp, dtype=np.float32)
    W1 = np.asarray(W1, dtype=np.float32)
    b1 = np.asarray(b1, dtype=np.float32)
    W2 = np.asarray(W2, dtype=np.float32)
    b2 = np.asarray(b2, dtype=np.float32)
    init_idx = np.asarray(init_idx)

    w1sb = np.ascontiguousarray(
        W1.reshape(NB, 128, D).transpose(1, 0, 2).reshape(128, NB * D))
    w2sb = np.ascontiguousarray(
        W2.reshape(NB, 128, D).transpose(1, 0, 2).reshape(128, NB * D))
    b1b = np.ascontiguousarray(np.broadcast_to(b1, (C, D)))
    b2b = np.ascontiguousarray(np.broadcast_to(b2, (C, D)))
    ident = np.eye(128, dtype=np.float32)

    maps = []
    for k in range(K):
        top, left, bot, right = boxes[k]
        xg = np.arange(W, dtype=np.float32) / np.float32(W) * (right - left) + left
        xg = np.clip(xg / np.float32(RAW_W - 1), 0.0, 1.0).astype(np.float32)
        yg = np.arange(H, dtype=np.float32) / np.float32(H) * (bot - top) + top
        yg = np.clip(yg / np.float32(RAW_H - 1), 0.0, 1.0).astype(np.float32)

        xb = (xg[:, None] * Wp[0][None, :] + bp[None, :]).astype(np.float32)
        wy = np.ascontiguousarray(np.broadcast_to(Wp[1], (128, D)))
        ygb = np.ascontiguousarray(np.broadcast_to(yg[None, :], (128, H)))
        mk = (mask[k] > 0).astype(np.float32)
        maskT = np.ascontiguousarray(mk.T)

        idx = init_idx[k].astype(np.int64)
        hr, wr = idx // W, idx % W
        fr = feat[k].reshape(NT, D)[idx]
        t1 = fr + yg[hr][:, None] * Wp[1][None, :]
        c0 = (t1 + xb[wr]).astype(np.float32)  # mask at init_idx is 1 by setup
        c0t = np.ascontiguousarray(
            c0.T.reshape(NB, 128, C).transpose(1, 0, 2).reshape(128, NB * C))
        ncn0 = np.ascontiguousarray(np.broadcast_to(
            (-0.5 * np.sum(c0.astype(np.float32) ** 2, axis=1)).astype(np.float32),
            (128, C)))

        cstbuf = np.zeros((128, CW), dtype=np.float32)
        cstbuf[:, COFF["xb"]:COFF["xb"] + D] = xb
        cstbuf[:, COFF["wy"]:COFF["wy"] + D] = wy
        cstbuf[:, COFF["ygb"]:COFF["ygb"] + H] = ygb
        cstbuf[:, COFF["mt"]:COFF["mt"] + H] = maskT
        cstbuf[:, COFF["c0t"]:COFF["c0t"] + NB * C] = c0t
        cstbuf[:, COFF["ncn0"]:COFF["ncn0"] + C] = ncn0
        cstbuf[:, COFF["ident"]:COFF["ident"] + 128] = ident
        cstbuf[0:C, COFF["b1b"]:COFF["b1b"] + D] = b1b
        cstbuf[0:C, COFF["b2b"]:COFF["b2b"] + D] = b2b
        cstbuf[:, COFF["w1"]:COFF["w1"] + NB * D] = w1sb
        cstbuf[:, COFF["w2"]:COFF["w2"] + NB * D] = w2sb
        maps.append({
            "feat": feat[k].reshape(NT, D),
            "consts": cstbuf,
        })
    return maps


def run(trace=False, **inputs):
    if "nc" not in _CACHE:
        _CACHE["nc"] = _build_program()
    nc = _CACHE["nc"]
    in_maps = _host_prep(
        inputs["feat"], inputs["mask"], inputs["boxes"], inputs["Wp"],
        inputs["bp"], inputs["W1"], inputs["b1"], inputs["W2"], inputs["b2"],
        inputs["init_idx"])
    res = run_bass_kernel_spmd(nc, in_maps, core_ids=list(range(K)),
                               trace=trace)
    out = np.stack([np.asarray(res.results[k]["out"]) for k in range(K)])
    return out.astype(np.float32), res


def kernel(**inputs):
    out, _ = run(trace=False, **inputs)
    return out


# revision 15
# speedup vs baseline: 1.0666x; 1.0666x over previous
"""vq_codebook Trainium2 kernel: pos-encode + masked k-means + proj MLP.

Sharding: pure data parallel over K=8 objects, one object per NeuronCore.

Per-core algorithm (all fp32 — k-means argmin margins are ~1e-5, bf16/fp22
token or distance compression empirically breaks the labels and the final
output; validated vs the jax reference at ~1e-6 rel err in numpy):

  pass 0:  stream feat tiles [128 tok, 768], build tokens = (feat + pos)*mask
           on DVE/GPSIMD/ACT, write tokens to HBM in BOTH layouts
           (natural [16384,768] and transposed [768,16384] via PE transposes),
           and run k-means iteration 1 fused (tiles already in SBUF).
  iters:   4 more k-means iterations; per tile: G = tokens @ c.T via
           6 stationary-tokensT matmuls -> psum [128,10]; q = G - cn2/2;
           DVE max + one-hot U (is_ge vs row max, masked); cluster sums via
           6 stationary-tokens matmuls accumulating psum [128d, 10c] over all
           tiles; counts via ones-stationary matmul.
  update:  counts broadcast (ones-row matmul), divide, select (empty clusters
           keep old centroid), cn2 via ones-col matmul + reduce.
  MLP:     h1 = gelu(c @ W1 + b1); out = h1 @ W2 + b2, PE transposes for h1.
"""

import numpy as np
from contextlib import ExitStack

import concourse.bass as bass
import concourse.bacc as bacc
import concourse.tile as tile
from concourse import mybir
from concourse.bass_utils import run_bass_kernel_spmd

import os
F32 = mybir.dt.float32
_GELU = (mybir.ActivationFunctionType.Identity
         if os.environ.get("KBDBG_NOGELU") else
         mybir.ActivationFunctionType.Gelu)
OP = mybir.AluOpType
AF = mybir.ActivationFunctionType

K, H, W, D, C, ITERS = 8, 128, 128, 768, 10, 5
NT = H * W            # 16384 tokens
NB = D // 128         # 6 d-blocks
NTILE = NT // 128     # 128 token tiles
RAW_H = RAW_W = 1024

COFF = {}
_off = 0
for _n, _w in [("xb", 768), ("wy", 768), ("ygb", 128), ("mt", 128),
               ("c0t", 60), ("ncn0", 10), ("ident", 128), ("c0n", 768),
               ("b1b", 768), ("b2b", 768), ("w1", 4608), ("w2", 4608)]:
    COFF[_n] = _off
    _off += _w
CW = _off

_CACHE = {}


def _build_program():
    nc = bacc.Bacc("TRN2", target_bir_lowering=False, debug=False, num_devices=K)

    feat = nc.dram_tensor("feat", [NT, D], F32, kind="ExternalInput").ap()
    cst_d = nc.dram_tensor("consts", [128, CW], F32, kind="ExternalInput").ap()
    out_d = nc.dram_tensor("out", [C, D], F32, kind="ExternalOutput").ap()

    tokN = nc.dram_tensor("tokn", [NT, D], F32).ap()
    tokT = nc.dram_tensor("tokt", [D, NT], F32).ap()
    # [d, t] -> [dlow, b, t] view for block DMA
    tokT3 = tokT.rearrange("(b p) t -> p b t", b=NB)

    with tile.TileContext(nc) as tc, ExitStack() as ctx:
        const = ctx.enter_context(tc.tile_pool(name="const", bufs=1))
        io = ctx.enter_context(tc.tile_pool(name="io", bufs=3))
        tmp = ctx.enter_context(tc.tile_pool(name="tmp", bufs=2))
        small = ctx.enter_context(tc.tile_pool(name="small", bufs=4))
        ctp = ctx.enter_context(tc.tile_pool(name="ctp", bufs=2))
        ps_tr = ctx.enter_context(tc.tile_pool(name="ps_tr", bufs=2, space="PSUM"))
        ps_g = ctx.enter_context(tc.tile_pool(name="ps_g", bufs=2, space="PSUM"))
        ps_acc = ctx.enter_context(tc.tile_pool(name="ps_acc", bufs=1, space="PSUM"))

        cst = const.tile([128, CW], F32, tag="cst")
        nc.sync.dma_start(cst[:, :], cst_d)
        xb = cst[:, COFF["xb"]:COFF["xb"] + D]
        wy = cst[:, COFF["wy"]:COFF["wy"] + D]
        ygb = cst[:, COFF["ygb"]:COFF["ygb"] + H]
        mT = cst[:, COFF["mt"]:COFF["mt"] + H]
        c0t = cst[:, COFF["c0t"]:COFF["c0t"] + NB * C]
        c0n = cst[0:C, COFF["c0n"]:COFF["c0n"] + D]
        ncn0 = cst[:, COFF["ncn0"]:COFF["ncn0"] + C]
        ident = cst[:, COFF["ident"]:COFF["ident"] + 128]
        b1b = cst[0:C, COFF["b1b"]:COFF["b1b"] + D]
        b2b = cst[0:C, COFF["b2b"]:COFF["b2b"] + D]
        w1 = cst[:, COFF["w1"]:COFF["w1"] + NB * D]
        w2 = cst[:, COFF["w2"]:COFF["w2"] + NB * D]
        ones_c = const.tile([128, 1], F32, tag="ones_c")
        nc.gpsimd.memset(ones_c[:, :], 1.0)
        ones_r = const.tile([1, 128], F32, tag="ones_r")
        nc.gpsimd.memset(ones_r[:, :], 1.0)

        def iter_tile_body(t_i, tokT_sb, tokN_sb, cT, ncnb, psS, psCnt, first):
            """One 128-token tile of one k-means iteration.

            tokT_sb: [128, NB*128] transposed blocks; tokN_sb: [128, D] natural.
            cT: [128, NB*C] centroids (d-blocks x c); ncnb: [128, C] = -cn2/2.
            """
            psG = ps_g.tile([128, 16], F32, tag="g")
            for b in range(NB):
                nc.tensor.matmul(
                    psG[:, 0:C],
                    tokT_sb[:, b * 128:(b + 1) * 128],
                    cT[:, b * C:(b + 1) * C],
                    start=(b == 0), stop=(b == NB - 1),
                )
            q = small.tile([128, C], F32, tag="q")
            nc.vector.tensor_tensor(q[:, :], psG[:, 0:C], ncnb[:, :], op=OP.add)
            mx = small.tile([128, 8], F32, tag="mx")
            nc.vector.max(mx[:, :], q[:, :])
            u = small.tile([128, C], F32, tag="u")
            nc.vector.tensor_scalar(
                u[:, :], q[:, :], mx[:, 0:1], mT[:, t_i:t_i + 1],
                op0=OP.is_ge, op1=OP.mult,
            )
            last = (t_i == NTILE - 1)
            nc.tensor.matmul(
                psS[0][:, :], u[:, :], tokN_sb[:, 0:D // 2],
                start=first, stop=False,
            )
            nc.tensor.matmul(
                psS[1][:, :], u[:, :], tokN_sb[:, D // 2:D],
                start=first, stop=last,
            )
            nc.tensor.matmul(
                psCnt[:, 0:1], u[:, :], ones_c[:, :],
                start=False, stop=last,
            )
            return u

        def iter_finish(psS, psCnt, cN_prev):
            """psS: 2x psum [C, D/2]; psCnt: psum [C, 16]; cN_prev: [C, D].
            Returns (cN, cT, ncnb): natural + transposed centroids, -cn2/2."""
            cb = small.tile([C, 1], F32, tag="cb")
            nc.vector.tensor_copy(cb[:, :], psCnt[:, 0:1])
            cmax = small.tile([C, 1], F32, tag="cmax")
            nc.vector.tensor_scalar(cmax[:, :], cb[:, :], 1.0, None, op0=OP.max)
            rcp = small.tile([C, 1], F32, tag="rcp")
            nc.vector.reciprocal(rcp[:, :], cmax[:, :])
            mb = small.tile([C, 1], F32, tag="mb")
            nc.vector.tensor_scalar(mb[:, :], cb[:, :], 0.5, None, op0=OP.is_ge)
            imb = small.tile([C, 1], F32, tag="imb")
            nc.vector.tensor_scalar(imb[:, :], mb[:, :], -1.0, 1.0,
                                    op0=OP.mult, op1=OP.add)
            cN = ctp.tile([C, D], F32, tag="cn")
            for hf in range(2):
                hs = slice(hf * (D // 2), (hf + 1) * (D // 2))
                tmp1 = small.tile([C, D // 2], F32, tag="tmp1")
                nc.vector.tensor_scalar(tmp1[:, :], psS[hf][:, :], rcp[:, :],
                                        mb[:, :], op0=OP.mult, op1=OP.mult)
                nc.vector.scalar_tensor_tensor(
                    cN[:, hs], cN_prev[:, hs], imb[:, :], tmp1[:, :],
                    op0=OP.mult, op1=OP.add)
            sq = tmp.tile([C, D], F32, tag="sq")
            nc.vector.tensor_tensor(sq[:, :], cN[:, :], cN[:, :], op=OP.mult)
            nr = small.tile([C, 1], F32, tag="nr")
            nc.vector.tensor_reduce(nr[:, :], sq[:, :],
                                    axis=mybir.AxisListType.X, op=OP.add)
            nr2 = small.tile([C, 1], F32, tag="nr2")
            nc.vector.tensor_scalar(nr2[:, :], nr[:, :], -0.5, None, op0=OP.mult)
            cT = ctp.tile([128, NB * C], F32, tag="cfin")
            for b in range(NB):
                psT = ps_g.tile([128, 16], F32, tag="g")
                nc.tensor.transpose(psT[:, 0:C], cN[:, b * 128:(b + 1) * 128],
                                    ident[0:C, 0:C])
                nc.vector.tensor_copy(cT[:, b * C:(b + 1) * C], psT[:, 0:C])
            psR = ps_g.tile([128, 16], F32, tag="g")
            nc.tensor.transpose(psR[0:1, 0:C], nr2[:, 0:1], ident[0:C, 0:C])
            nrr = small.tile([1, C], F32, tag="nrr")
            nc.vector.tensor_copy(nrr[:, :], psR[0:1, 0:C])
            psB2 = ps_g.tile([128, 16], F32, tag="g")
            nc.tensor.matmul(psB2[:, 0:C], ones_r[:, :], nrr[:, :],
                             start=True, stop=True)
            ncnb = ctp.tile([128, C], F32, tag="ncnb")
            nc.vector.tensor_copy(ncnb[:, :], psB2[:, 0:C])
            return cN, cT, ncnb

        # ---------------- pass 0 + fused k-means iter 1 ----------------
        psA = ps_acc.tile([C, 512], F32, tag="acc0")
        psS1 = ps_acc.tile([C, D // 2], F32, tag="acc1")
        psS = [psA[:, 0:D // 2], psS1]
        psCnt = psA[:, D // 2:D // 2 + 16]
        for t_i in range(NTILE):
            ft = io.tile([128, D], F32, tag="ft")
            nc.sync.dma_start(ft[:, :], feat[t_i * 128:(t_i + 1) * 128, :])
            t1 = tmp.tile([128, D], F32, tag="t1")
            nc.vector.scalar_tensor_tensor(
                t1[:, :], wy, ygb[:, t_i:t_i + 1], ft[:, :],
                op0=OP.mult, op1=OP.add,
            )
            t2 = tmp.tile([128, D], F32, tag="t2")
            nc.gpsimd.tensor_tensor(t2[:, :], t1[:, :], xb, op=OP.add)
            tok = io.tile([128, D], F32, tag="tok")
            nc.scalar.mul(tok[:, :], t2[:, :], mT[:, t_i:t_i + 1])
            nc.sync.dma_start(tokN[t_i * 128:(t_i + 1) * 128, :], tok[:, :])
            ptr = ps_tr.tile([128, D], F32, tag="tr")
            tokT_sb = io.tile([128, D], F32, tag="tokt")
            for b in range(NB):
                sl = slice(b * 128, (b + 1) * 128)
                nc.tensor.transpose(ptr[:, sl], tok[:, sl], ident)
                if b % 2 == 0:
                    nc.vector.tensor_copy(tokT_sb[:, sl], ptr[:, sl])
                else:
                    nc.scalar.copy(tokT_sb[:, sl], ptr[:, sl])
            nc.sync.dma_start(
                tokT3[:, :, t_i * 128:(t_i + 1) * 128],
                tokT_sb[:, :].rearrange("p (b t) -> p b t", b=NB),
            )
            iter_tile_body(t_i, tokT_sb, tok, c0t, ncn0, psS, psCnt,
                           first=(t_i == 0))
        cN, cT, ncnb = iter_finish(psS, psCnt, c0n)

        # ---------------- k-means iterations 2..ITERS ----------------
        for it in range(1, ITERS):
            psA = ps_acc.tile([C, 512], F32, tag="acc0")
            psS1 = ps_acc.tile([C, D // 2], F32, tag="acc1")
            psS = [psA[:, 0:D // 2], psS1]
            psCnt = psA[:, D // 2:D // 2 + 16]
            for t_i in range(0, NTILE, 2):
                pairN = io.tile([128, 2 * D], F32, tag="tok")
                nc.sync.dma_start(
                    pairN[:, :].rearrange("p (i e) -> p i e", i=2),
                    feat_pair_src(tokN, t_i),
                )
                pairT = io.tile([128, 2 * D], F32, tag="tokt")
                for i in range(2):
                    nc.sync.dma_start(
                        pairT[:, i * D:(i + 1) * D]
                            .rearrange("p (b t) -> p b t", b=NB),
                        tokT3[:, :, (t_i + i) * 128:(t_i + i + 1) * 128],
                    )
                for i in range(2):
                    iter_tile_body(
                        t_i + i,
                        pairT[:, i * D:(i + 1) * D],
                        pairN[:, i * D:(i + 1) * D],
                        cT, ncnb, psS, psCnt,
                        first=(t_i + i == 0),
                    )
            cN, cT, ncnb = iter_finish(psS, psCnt, cN)

        # ---------------- projection MLP ----------------
        h1 = tmp.tile([C, D], F32, tag="h1")
        for half in range(2):
            hs = slice(half * (D // 2), (half + 1) * (D // 2))
            psH = ps_g.tile([C, D // 2], F32, tag="g")
            for b in range(NB):
                nc.tensor.matmul(
                    psH[:, :], cT[:, b * C:(b + 1) * C],
                    w1[:, b * D + half * (D // 2): b * D + (half + 1) * (D // 2)],
                    start=(b == 0), stop=(b == NB - 1),
                )
            hb = tmp.tile([C, D // 2], F32, tag="hb")
            nc.vector.tensor_tensor(hb[:, :], psH[:, :], b1b[:, hs], op=OP.add)
            nc.scalar.activation(h1[:, hs], hb[:, :], _GELU)
        h1t = tmp.tile([128, NB * C], F32, tag="h1t")
        for b in range(NB):
            psT = ps_g.tile([128, 16], F32, tag="g")
            nc.tensor.transpose(psT[:, 0:C], h1[:, b * 128:(b + 1) * 128],
                                ident[0:C, 0:C])
            nc.vector.tensor_copy(h1t[:, b * C:(b + 1) * C], psT[:, 0:C])
        osb = tmp.tile([C, D], F32, tag="osb")
        for half in range(2):
            hs = slice(half * (D // 2), (half + 1) * (D // 2))
            psO = ps_g.tile([C, D // 2], F32, tag="g")
            for b in range(NB):
                nc.tensor.matmul(
                    psO[:, :], h1t[:, b * C:(b + 1) * C],
                    w2[:, b * D + half * (D // 2): b * D + (half + 1) * (D // 2)],
                    start=(b == 0), stop=(b == NB - 1),
                )
            nc.vector.tensor_tensor(osb[:, hs], psO[:, :], b2b[:, hs], op=OP.add)
        nc.sync.dma_start(out_d, osb[:, :])

    nc.compile()
    return nc


def feat_pair_src(tokN, t_i):
    return (tokN[t_i * 128:(t_i + 2) * 128, :]
            .rearrange("(i p) e -> p i e", i=2))


def _host_prep(feat, mask, boxes, Wp, bp, W1, b1, W2, b2, init_idx):
    """Per-core input dicts. All small O(K*(H+W)*D) prep; feat is viewed."""
    feat = np.ascontiguousarray(np.asarray(feat, dtype=np.float32))
    mask = np.asarray(mask, dtype=np.float32)
    boxes = np.asarray(boxes, dtype=np.float32)
    Wp = np.asarray(Wp, dtype=np.float32)
    bp = np.asarray(bp, dtype=np.float32)
    W1 = np.asarray(W1, dtype=np.float32)
    b1 = np.asarray(b1, dtype=np.float32)
    W2 = np.asarray(W2, dtype=np.float32)
    b2 = np.asarray(b2, dtype=np.float32)
    init_idx = np.asarray(init_idx)

    w1sb = np.ascontiguousarray(
        W1.reshape(NB, 128, D).transpose(1, 0, 2).reshape(128, NB * D))
    w2sb = np.ascontiguousarray(
        W2.reshape(NB, 128, D).transpose(1, 0, 2).reshape(128, NB * D))
    b1b = np.ascontiguousarray(np.broadcast_to(b1, (C, D)))
    b2b = np.ascontiguousarray(np.broadcast_to(b2, (C, D)))
    ident = np.eye(128, dtype=np.float32)

    maps = []
    for k in range(K):
        top, left, bot, right = boxes[k]
        xg = np.arange(W, dtype=np.float32) / np.float32(W) * (right - left) + left
        xg = np.clip(xg / np.float32(RAW_W - 1), 0.0, 1.0).astype(np.float32)
        yg = np.arange(H, dtype=np.float32) / np.float32(H) * (bot - top) + top
        yg = np.clip(yg / np.float32(RAW_H - 1), 0.0, 1.0).astype(np.float32)

        xb = (xg[:, None] * Wp[0][None, :] + bp[None, :]).astype(np.float32)
        wy = np.ascontiguousarray(np.broadcast_to(Wp[1], (128, D)))
        ygb = np.ascontiguousarray(np.broadcast_to(yg[None, :], (128, H)))
        mk = (mask[k] > 0).astype(np.float32)
        maskT = np.ascontiguousarray(mk.T)

        idx = init_idx[k].astype(np.int64)
        hr, wr = idx // W, idx % W
        fr = feat[k].reshape(NT, D)[idx]
        t1 = fr + yg[hr][:, None] * Wp[1][None, :]
        c0 = (t1 + xb[wr]).astype(np.float32)  # mask at init_idx is 1 by setup
        c0t = np.ascontiguousarray(
            c0.T.reshape(NB, 128, C).transpose(1, 0, 2).reshape(128, NB * C))
        ncn0 = np.ascontiguousarray(np.broadcast_to(
            (-0.5 * np.sum(c0.astype(np.float32) ** 2, axis=1)).astype(np.float32),
            (128, C)))

        cstbuf = np.zeros((128, CW), dtype=np.float32)
        cstbuf[:, COFF["xb"]:COFF["xb"] + D] = xb
        cstbuf[:, COFF["wy"]:COFF["wy"] + D] = wy
        cstbuf[:, COFF["ygb"]:COFF["ygb"] + H] = ygb
        cstbuf[:, COFF["mt"]:COFF["mt"] + H] = maskT
        cstbuf[:, COFF["c0t"]:COFF["c0t"] + NB * C] = c0t
        cstbuf[0:C, COFF["c0n"]:COFF["c0n"] + D] = c0
        cstbuf[:, COFF["ncn0"]:COFF["ncn0"] + C] = ncn0
        cstbuf[:, COFF["ident"]:COFF["ident"] + 128] = ident
        cstbuf[0:C, COFF["b1b"]:COFF["b1b"] + D] = b1b
        cstbuf[0:C, COFF["b2b"]:COFF["b2b"] + D] = b2b
        cstbuf[:, COFF["w1"]:COFF["w1"] + NB * D] = w1sb
        cstbuf[:, COFF["w2"]:COFF["w2"] + NB * D] = w2sb
        maps.append({
            "feat": feat[k].reshape(NT, D),
            "consts": cstbuf,
        })
    return maps


def run(trace=False, **inputs):
    if "nc" not in _CACHE:
        _CACHE["nc"] = _build_program()
    nc = _CACHE["nc"]
    in_maps = _host_prep(
        inputs["feat"], inputs["mask"], inputs["boxes"], inputs["Wp"],
        inputs["bp"], inputs["W1"], inputs["b1"], inputs["W2"], inputs["b2"],
        inputs["init_idx"])
    res = run_bass_kernel_spmd(nc, in_maps, core_ids=list(range(K)),
                               trace=trace)
    out = np.stack([np.asarray(res.results[k]["out"]) for k in range(K)])
    return out.astype(np.float32), res


def kernel(**inputs):
    out, _ = run(trace=False, **inputs)
    return out


# revision 16
# speedup vs baseline: 1.4324x; 1.3430x over previous
"""vq_codebook Trainium2 kernel: pos-encode + masked k-means + proj MLP.

Sharding: pure data parallel over K=8 objects, one object per NeuronCore.

Per-core algorithm (all fp32 — k-means argmin margins are ~1e-5, bf16/fp22
token or distance compression empirically breaks the labels and the final
output; validated vs the jax reference at ~1e-6 rel err in numpy):

  pass 0:  stream feat tiles [128 tok, 768], build tokens = (feat + pos)*mask
           on DVE/GPSIMD/ACT, write tokens to HBM in BOTH layouts
           (natural [16384,768] and transposed [768,16384] via PE transposes),
           and run k-means iteration 1 fused (tiles already in SBUF).
  iters:   4 more k-means iterations; per tile: G = tokens @ c.T via
           6 stationary-tokensT matmuls -> psum [128,10]; q = G - cn2/2;
           DVE max + one-hot U (is_ge vs row max, masked); cluster sums via
           6 stationary-tokens matmuls accumulating psum [128d, 10c] over all
           tiles; counts via ones-stationary matmul.
  update:  counts broadcast (ones-row matmul), divide, select (empty clusters
           keep old centroid), cn2 via ones-col matmul + reduce.
  MLP:     h1 = gelu(c @ W1 + b1); out = h1 @ W2 + b2, PE transposes for h1.
"""

import numpy as np
from contextlib import ExitStack

import concourse.bass as bass
import concourse.bacc as bacc
import concourse.tile as tile
from concourse import mybir
from concourse.bass_utils import run_bass_kernel_spmd

import os
F32 = mybir.dt.float32
_GELU = (mybir.ActivationFunctionType.Identity
         if os.environ.get("KBDBG_NOGELU") else
         mybir.ActivationFunctionType.Gelu)
OP = mybir.AluOpType
AF = mybir.ActivationFunctionType

K, H, W, D, C, ITERS = 8, 128, 128, 768, 10, 5
NT = H * W            # 16384 tokens
NB = D // 128         # 6 d-blocks
NTILE = NT // 128     # 128 token tiles
RAW_H = RAW_W = 1024

COFF = {}
_off = 0
for _n, _w in [("xb", 768), ("wy", 768), ("ygb", 128), ("mt", 128),
               ("c0t", 60), ("ncn0", 10), ("ident", 128), ("c0n", 768),
               ("b1b", 768), ("b2b", 768), ("w1", 4608), ("w2", 4608)]:
    COFF[_n] = _off
    _off += _w
CW = _off

_CACHE = {}


def _build_program():
    nc = bacc.Bacc("TRN2", target_bir_lowering=False, debug=False, num_devices=K)

    feat = nc.dram_tensor("feat", [NT, D], F32, kind="ExternalInput").ap()
    cst_d = nc.dram_tensor("consts", [128, CW], F32, kind="ExternalInput").ap()
    out_d = nc.dram_tensor("out", [C, D], F32, kind="ExternalOutput").ap()

    tokN = nc.dram_tensor("tokn", [NT, D], F32).ap()
    tokT = nc.dram_tensor("tokt", [D, NT], F32).ap()
    # [d, t] -> [dlow, b, t] view for block DMA
    tokT3 = tokT.rearrange("(b p) t -> p b t", b=NB)

    with tile.TileContext(nc) as tc, ExitStack() as ctx:
        const = ctx.enter_context(tc.tile_pool(name="const", bufs=1))
        io = ctx.enter_context(tc.tile_pool(name="io", bufs=3))
        tmp = ctx.enter_context(tc.tile_pool(name="tmp", bufs=2))
        small = ctx.enter_context(tc.tile_pool(name="small", bufs=4))
        ctp = ctx.enter_context(tc.tile_pool(name="ctp", bufs=2))
        ps_tr = ctx.enter_context(tc.tile_pool(name="ps_tr", bufs=1, space="PSUM"))
        ps_qtr = ctx.enter_context(tc.tile_pool(name="ps_qtr", bufs=2, space="PSUM"))
        ps_g = ctx.enter_context(tc.tile_pool(name="ps_g", bufs=2, space="PSUM"))
        ps_acc = ctx.enter_context(tc.tile_pool(name="ps_acc", bufs=1, space="PSUM"))

        cst = const.tile([128, CW], F32, tag="cst")
        nc.sync.dma_start(cst[:, :], cst_d)
        xb = cst[:, COFF["xb"]:COFF["xb"] + D]
        wy = cst[:, COFF["wy"]:COFF["wy"] + D]
        ygb = cst[:, COFF["ygb"]:COFF["ygb"] + H]
        mT = cst[:, COFF["mt"]:COFF["mt"] + H]
        c0t = cst[:, COFF["c0t"]:COFF["c0t"] + NB * C]
        c0n = cst[0:C, COFF["c0n"]:COFF["c0n"] + D]
        ncn0c = cst[0:C, COFF["ncn0"]:COFF["ncn0"] + 1]
        ident = cst[:, COFF["ident"]:COFF["ident"] + 128]
        b1b = cst[0:C, COFF["b1b"]:COFF["b1b"] + D]
        b2b = cst[0:C, COFF["b2b"]:COFF["b2b"] + D]
        w1 = cst[:, COFF["w1"]:COFF["w1"] + NB * D]
        w2 = cst[:, COFF["w2"]:COFF["w2"] + NB * D]
        ones_c = const.tile([128, 1], F32, tag="ones_c")
        nc.gpsimd.memset(ones_c[:, :], 1.0)
        ones_r = const.tile([1, 128], F32, tag="ones_r")
        nc.gpsimd.memset(ones_r[:, :], 1.0)

        def group_labels(g_i, tokTg, cT, ncn_col):
            """tokTg: [128, NB*512] (block b at cols b*512); cT: [128, NB*C];
            ncn_col: [C, 1] = -0.5*||c||^2. Returns 4 one-hot U tiles [128, C]."""
            psGp = ps_g.tile([C, 512], F32, tag="g")
            for b in range(NB):
                nc.tensor.matmul(
                    psGp[:, :],
                    cT[:, b * C:(b + 1) * C],
                    tokTg[:, b * 512:(b + 1) * 512],
                    start=(b == 0), stop=(b == NB - 1),
                )
            qg = tmp.tile([C, 512], F32, tag="qg")
            nc.vector.tensor_scalar(qg[:, :], psGp[:, :], ncn_col, None,
                                    op0=OP.add)
            us = []
            for i in range(4):
                t_i = g_i * 4 + i
                psQ = ps_qtr.tile([128, 16], F32, tag="qtr")
                nc.tensor.transpose(psQ[:, 0:C], qg[:, i * 128:(i + 1) * 128],
                                    ident[0:C, 0:C])
                q = small.tile([128, C], F32, tag="q")
                nc.vector.tensor_copy(q[:, :], psQ[:, 0:C])
                mx = small.tile([128, 8], F32, tag="mx")
                nc.vector.max(mx[:, :], q[:, :])
                u = small.tile([128, C], F32, tag="u")
                nc.vector.tensor_scalar(
                    u[:, :], q[:, :], mx[:, 0:1], mT[:, t_i:t_i + 1],
                    op0=OP.is_ge, op1=OP.mult,
                )
                us.append(u)
            return us

        def group_sums(g_i, us, tokNg, psS, psCnt):
            for i in range(4):
                t_i = g_i * 4 + i
                first = (t_i == 0)
                last = (t_i == NTILE - 1)
                nc.tensor.matmul(psS[0][:, :], us[i][:, :],
                                 tokNg[:, i * D:i * D + D // 2],
                                 start=first, stop=False)
                nc.tensor.matmul(psS[1][:, :], us[i][:, :],
                                 tokNg[:, i * D + D // 2:(i + 1) * D],
                                 start=first, stop=last)
                nc.tensor.matmul(psCnt[:, 0:1], us[i][:, :], ones_c[:, :],
                                 start=False, stop=last)

        def iter_finish(psS, psCnt, cN_prev):
            """psS: 2x psum [C, D/2]; psCnt: psum [C, 16]; cN_prev: [C, D].
            Returns (cN, cT, ncnb): natural + transposed centroids, -cn2/2."""
            cb = small.tile([C, 1], F32, tag="cb")
            nc.vector.tensor_copy(cb[:, :], psCnt[:, 0:1])
            cmax = small.tile([C, 1], F32, tag="cmax")
            nc.vector.tensor_scalar(cmax[:, :], cb[:, :], 1.0, None, op0=OP.max)
            rcp = small.tile([C, 1], F32, tag="rcp")
            nc.vector.reciprocal(rcp[:, :], cmax[:, :])
            mb = small.tile([C, 1], F32, tag="mb")
            nc.vector.tensor_scalar(mb[:, :], cb[:, :], 0.5, None, op0=OP.is_ge)
            imb = small.tile([C, 1], F32, tag="imb")
            nc.vector.tensor_scalar(imb[:, :], mb[:, :], -1.0, 1.0,
                                    op0=OP.mult, op1=OP.add)
            cN = ctp.tile([C, D], F32, tag="cn")
            for hf in range(2):
                hs = slice(hf * (D // 2), (hf + 1) * (D // 2))
                tmp1 = small.tile([C, D // 2], F32, tag="tmp1")
                nc.vector.tensor_scalar(tmp1[:, :], psS[hf][:, :], rcp[:, :],
                                        mb[:, :], op0=OP.mult, op1=OP.mult)
                nc.vector.scalar_tensor_tensor(
                    cN[:, hs], cN_prev[:, hs], imb[:, :], tmp1[:, :],
                    op0=OP.mult, op1=OP.add)
            sq = tmp.tile([C, D], F32, tag="sq")
            nc.vector.tensor_tensor(sq[:, :], cN[:, :], cN[:, :], op=OP.mult)
            nr = small.tile([C, 1], F32, tag="nr")
            nc.vector.tensor_reduce(nr[:, :], sq[:, :],
                                    axis=mybir.AxisListType.X, op=OP.add)
            nr2 = small.tile([C, 1], F32, tag="nr2")
            nc.vector.tensor_scalar(nr2[:, :], nr[:, :], -0.5, None, op0=OP.mult)
            cT = ctp.tile([128, NB * C], F32, tag="cfin")
            for b in range(NB):
                psT = ps_g.tile([128, 16], F32, tag="g")
                nc.tensor.transpose(psT[:, 0:C], cN[:, b * 128:(b + 1) * 128],
                                    ident[0:C, 0:C])
                nc.vector.tensor_copy(cT[:, b * C:(b + 1) * C], psT[:, 0:C])
            return cN, cT, nr2

        # ---------------- pass 0 + fused k-means iter 1 ----------------
        psA = ps_acc.tile([C, 512], F32, tag="acc0")
        psS1 = ps_acc.tile([C, D // 2], F32, tag="acc1")
        psS = [psA[:, 0:D // 2], psS1]
        psCnt = psA[:, D // 2:D // 2 + 16]
        for g_i in range(NTILE // 4):
            tokNg = io.tile([128, 4 * D], F32, tag="tok")
            tokTg = io.tile([128, NB * 512], F32, tag="tokt")
            for i in range(4):
                t_i = g_i * 4 + i
                ft = io.tile([128, D], F32, tag="ft")
                nc.sync.dma_start(ft[:, :], feat[t_i * 128:(t_i + 1) * 128, :])
                t1 = tmp.tile([128, D], F32, tag="t1")
                nc.vector.scalar_tensor_tensor(
                    t1[:, :], wy, ygb[:, t_i:t_i + 1], ft[:, :],
                    op0=OP.mult, op1=OP.add,
                )
                t2 = tmp.tile([128, D], F32, tag="t2")
                nc.gpsimd.tensor_tensor(t2[:, :], t1[:, :], xb, op=OP.add)
                nc.scalar.mul(tokNg[:, i * D:(i + 1) * D], t2[:, :],
                              mT[:, t_i:t_i + 1])
                ptr = ps_tr.tile([128, D], F32, tag="tr")
                for b in range(NB):
                    nc.tensor.transpose(
                        ptr[:, b * 128:(b + 1) * 128],
                        tokNg[:, i * D + b * 128:i * D + (b + 1) * 128],
                        ident)
                for b in range(NB):
                    dst = tokTg[:, b * 512 + i * 128:b * 512 + (i + 1) * 128]
                    if b % 2 == 0:
                        nc.vector.tensor_copy(dst, ptr[:, b * 128:(b + 1) * 128])
                    else:
                        nc.scalar.copy(dst, ptr[:, b * 128:(b + 1) * 128])
            nc.sync.dma_start(
                tokN[g_i * 512:(g_i + 1) * 512, :]
                    .rearrange("(i p) e -> p i e", i=4),
                tokNg[:, :].rearrange("p (i e) -> p i e", i=4))
            nc.sync.dma_start(
                tokT3[:, :, g_i * 512:(g_i + 1) * 512],
                tokTg[:, :].rearrange("p (b t) -> p b t", b=NB))
            us = group_labels(g_i, tokTg, c0t, ncn0c)
            group_sums(g_i, us, tokNg, psS, psCnt)
        cN, cT, ncn = iter_finish(psS, psCnt, c0n)

        # ---------------- k-means iterations 2..ITERS ----------------
        for it in range(1, ITERS):
            psA = ps_acc.tile([C, 512], F32, tag="acc0")
            psS1 = ps_acc.tile([C, D // 2], F32, tag="acc1")
            psS = [psA[:, 0:D // 2], psS1]
            psCnt = psA[:, D // 2:D // 2 + 16]
            for g_i in range(NTILE // 4):
                tokNg = io.tile([128, 4 * D], F32, tag="tok")
                nc.sync.dma_start(
                    tokNg[:, :].rearrange("p (i e) -> p i e", i=4),
                    tokN[g_i * 512:(g_i + 1) * 512, :]
                        .rearrange("(i p) e -> p i e", i=4))
                tokTg = io.tile([128, NB * 512], F32, tag="tokt")
                nc.sync.dma_start(
                    tokTg[:, :].rearrange("p (b t) -> p b t", b=NB),
                    tokT3[:, :, g_i * 512:(g_i + 1) * 512])
                us = group_labels(g_i, tokTg, cT, ncn)
                group_sums(g_i, us, tokNg, psS, psCnt)
            cN, cT, ncn = iter_finish(psS, psCnt, cN)

        # ---------------- projection MLP ----------------
        h1 = tmp.tile([C, D], F32, tag="h1")
        for half in range(2):
            hs = slice(half * (D // 2), (half + 1) * (D // 2))
            psH = ps_g.tile([C, D // 2], F32, tag="g")
            for b in range(NB):
                nc.tensor.matmul(
                    psH[:, :], cT[:, b * C:(b + 1) * C],
                    w1[:, b * D + half * (D // 2): b * D + (half + 1) * (D // 2)],
                    start=(b == 0), stop=(b == NB - 1),
                )
            hb = tmp.tile([C, D // 2], F32, tag="hb")
            nc.vector.tensor_tensor(hb[:, :], psH[:, :], b1b[:, hs], op=OP.add)
            nc.scalar.activation(h1[:, hs], hb[:, :], _GELU)
        h1t = tmp.tile([128, NB * C], F32, tag="h1t")
        for b in range(NB):
            psT = ps_g.tile([128, 16], F32, tag="g")
            nc.tensor.transpose(psT[:, 0:C], h1[:, b * 128:(b + 1) * 128],
                                ident[0:C, 0:C])
            nc.vector.tensor_copy(h1t[:, b * C:(b + 1) * C], psT[:, 0:C])
        osb = tmp.tile([C, D], F32, tag="osb")
        for half in range(2):
            hs = slice(half * (D // 2), (half + 1) * (D // 2))
            psO = ps_g.tile([C, D // 2], F32, tag="g")
            for b in range(NB):
                nc.tensor.matmul(
                    psO[:, :], h1t[:, b * C:(b + 1) * C],
                    w2[:, b * D + half * (D // 2): b * D + (half + 1) * (D // 2)],
                    start=(b == 0), stop=(b == NB - 1),
                )
            nc.vector.tensor_tensor(osb[:, hs], psO[:, :], b2b[:, hs], op=OP.add)
        nc.sync.dma_start(out_d, osb[:, :])

    nc.compile()
    return nc


def feat_pair_src(tokN, t_i):
    return (tokN[t_i * 128:(t_i + 2) * 128, :]
            .rearrange("(i p) e -> p i e", i=2))


def _host_prep(feat, mask, boxes, Wp, bp, W1, b1, W2, b2, init_idx):
    """Per-core input dicts. All small O(K*(H+W)*D) prep; feat is viewed."""
    feat = np.ascontiguousarray(np.asarray(feat, dtype=np.float32))
    mask = np.asarray(mask, dtype=np.float32)
    boxes = np.asarray(boxes, dtype=np.float32)
    Wp = np.asarray(Wp, dtype=np.float32)
    bp = np.asarray(bp, dtype=np.float32)
    W1 = np.asarray(W1, dtype=np.float32)
    b1 = np.asarray(b1, dtype=np.float32)
    W2 = np.asarray(W2, dtype=np.float32)
    b2 = np.asarray(b2, dtype=np.float32)
    init_idx = np.asarray(init_idx)

    w1sb = np.ascontiguousarray(
        W1.reshape(NB, 128, D).transpose(1, 0, 2).reshape(128, NB * D))
    w2sb = np.ascontiguousarray(
        W2.reshape(NB, 128, D).transpose(1, 0, 2).reshape(128, NB * D))
    b1b = np.ascontiguousarray(np.broadcast_to(b1, (C, D)))
    b2b = np.ascontiguousarray(np.broadcast_to(b2, (C, D)))
    ident = np.eye(128, dtype=np.float32)

    maps = []
    for k in range(K):
        top, left, bot, right = boxes[k]
        xg = np.arange(W, dtype=np.float32) / np.float32(W) * (right - left) + left
        xg = np.clip(xg / np.float32(RAW_W - 1), 0.0, 1.0).astype(np.float32)
        yg = np.arange(H, dtype=np.float32) / np.float32(H) * (bot - top) + top
        yg = np.clip(yg / np.float32(RAW_H - 1), 0.0, 1.0).astype(np.float32)

        xb = (xg[:, None] * Wp[0][None, :] + bp[None, :]).astype(np.float32)
        wy = np.ascontiguousarray(np.broadcast_to(Wp[1], (128, D)))
        ygb = np.ascontiguousarray(np.broadcast_to(yg[None, :], (128, H)))
        mk = (mask[k] > 0).astype(np.float32)
        maskT = np.ascontiguousarray(mk.T)

        idx = init_idx[k].astype(np.int64)
        hr, wr = idx // W, idx % W
        fr = feat[k].reshape(NT, D)[idx]
        t1 = fr + yg[hr][:, None] * Wp[1][None, :]
        c0 = (t1 + xb[wr]).astype(np.float32)  # mask at init_idx is 1 by setup
        c0t = np.ascontiguousarray(
            c0.T.reshape(NB, 128, C).transpose(1, 0, 2).reshape(128, NB * C))
        ncn0c = (-0.5 * np.sum(c0.astype(np.float32) ** 2,
                                axis=1)).astype(np.float32)[:, None]

        cstbuf = np.zeros((128, CW), dtype=np.float32)
        cstbuf[:, COFF["xb"]:COFF["xb"] + D] = xb
        cstbuf[:, COFF["wy"]:COFF["wy"] + D] = wy
        cstbuf[:, COFF["ygb"]:COFF["ygb"] + H] = ygb
        cstbuf[:, COFF["mt"]:COFF["mt"] + H] = maskT
        cstbuf[:, COFF["c0t"]:COFF["c0t"] + NB * C] = c0t
        cstbuf[0:C, COFF["c0n"]:COFF["c0n"] + D] = c0
        cstbuf[0:C, COFF["ncn0"]:COFF["ncn0"] + 1] = ncn0c
        cstbuf[:, COFF["ident"]:COFF["ident"] + 128] = ident
        cstbuf[0:C, COFF["b1b"]:COFF["b1b"] + D] = b1b
        cstbuf[0:C, COFF["b2b"]:COFF["b2b"] + D] = b2b
        cstbuf[:, COFF["w1"]:COFF["w1"] + NB * D] = w1sb
        cstbuf[:, COFF["w2"]:COFF["w2"] + NB * D] = w2sb
        maps.append({
            "feat": feat[k].reshape(NT, D),
            "consts": cstbuf,
        })
    return maps


def run(trace=False, **inputs):
    if "nc" not in _CACHE:
        _CACHE["nc"] = _build_program()
    nc = _CACHE["nc"]
    in_maps = _host_prep(
        inputs["feat"], inputs["mask"], inputs["boxes"], inputs["Wp"],
        inputs["bp"], inputs["W1"], inputs["b1"], inputs["W2"], inputs["b2"],
        inputs["init_idx"])
    res = run_bass_kernel_spmd(nc, in_maps, core_ids=list(range(K)),
                               trace=trace)
    out = np.stack([np.asarray(res.results[k]["out"]) for k in range(K)])
    return out.astype(np.float32), res


def kernel(**inputs):
    out, _ = run(trace=False, **inputs)
    return out


# revision 19
# speedup vs baseline: 1.4919x; 1.0415x over previous
"""vq_codebook Trainium2 kernel: pos-encode + masked k-means + proj MLP.

Sharding: pure data parallel over K=8 objects, one object per NeuronCore.

Per-core algorithm (all fp32 — k-means argmin margins are ~1e-5, bf16/fp22
token or distance compression empirically breaks the labels and the final
output; validated vs the jax reference at ~1e-6 rel err in numpy):

  pass 0:  stream feat tiles [128 tok, 768], build tokens = (feat + pos)*mask
           on DVE/GPSIMD/ACT, write tokens to HBM in BOTH layouts
           (natural [16384,768] and transposed [768,16384] via PE transposes),
           and run k-means iteration 1 fused (tiles already in SBUF).
  iters:   4 more k-means iterations; per tile: G = tokens @ c.T via
           6 stationary-tokensT matmuls -> psum [128,10]; q = G - cn2/2;
           DVE max + one-hot U (is_ge vs row max, masked); cluster sums via
           6 stationary-tokens matmuls accumulating psum [128d, 10c] over all
           tiles; counts via ones-stationary matmul.
  update:  counts broadcast (ones-row matmul), divide, select (empty clusters
           keep old centroid), cn2 via ones-col matmul + reduce.
  MLP:     h1 = gelu(c @ W1 + b1); out = h1 @ W2 + b2, PE transposes for h1.
"""

import numpy as np
from contextlib import ExitStack

import concourse.bass as bass
import concourse.bacc as bacc
import concourse.tile as tile
from concourse import mybir
from concourse.bass_utils import run_bass_kernel_spmd

import os
F32 = mybir.dt.float32
F16 = mybir.dt.float16
_GELU = (mybir.ActivationFunctionType.Identity
         if os.environ.get("KBDBG_NOGELU") else
         mybir.ActivationFunctionType.Gelu)
OP = mybir.AluOpType
AF = mybir.ActivationFunctionType

K, H, W, D, C, ITERS = 8, 128, 128, 768, 10, 5
NT = H * W            # 16384 tokens
NB = D // 128         # 6 d-blocks
NTILE = NT // 128     # 128 token tiles
RAW_H = RAW_W = 1024

COFF = {}
_off = 0
for _n, _w in [("xb", 768), ("wy", 768), ("ygb", 128), ("mt", 128),
               ("c0t", 60), ("ncn0", 10), ("ident", 128), ("c0n", 768),
               ("b1b", 768), ("b2b", 768), ("w1", 4608), ("w2", 4608)]:
    COFF[_n] = _off
    _off += _w
CW = _off

_CACHE = {}


def _build_program():
    nc = bacc.Bacc("TRN2", target_bir_lowering=False, debug=False, num_devices=K)

    feat = nc.dram_tensor("feat", [NT, D], F32, kind="ExternalInput").ap()
    cst_d = nc.dram_tensor("consts", [128, CW], F32, kind="ExternalInput").ap()
    out_d = nc.dram_tensor("out", [C, D], F32, kind="ExternalOutput").ap()

    tokNh = nc.dram_tensor("toknh", [NT, D], F16).ap()
    tokNl = nc.dram_tensor("toknl", [NT, D], F16).ap()
    tokTh = nc.dram_tensor("tokth", [D, NT], F16).ap()
    tokTl = nc.dram_tensor("toktl", [D, NT], F16).ap()
    # [d, t] -> [dlow, b, t] views for block DMA
    tokTh3 = tokTh.rearrange("(b p) t -> p b t", b=NB)
    tokTl3 = tokTl.rearrange("(b p) t -> p b t", b=NB)

    with tile.TileContext(nc) as tc, ExitStack() as ctx:
        const = ctx.enter_context(tc.tile_pool(name="const", bufs=1))
        io = ctx.enter_context(tc.tile_pool(name="io", bufs=3))
        tmp = ctx.enter_context(tc.tile_pool(name="tmp", bufs=2))
        small = ctx.enter_context(tc.tile_pool(name="small", bufs=4))
        ctp = ctx.enter_context(tc.tile_pool(name="ctp", bufs=2))
        ps_tr = ctx.enter_context(tc.tile_pool(name="ps_tr", bufs=1, space="PSUM"))
        ps_qtr = ctx.enter_context(tc.tile_pool(name="ps_qtr", bufs=2, space="PSUM"))
        ps_g = ctx.enter_context(tc.tile_pool(name="ps_g", bufs=2, space="PSUM"))
        ps_acc = ctx.enter_context(tc.tile_pool(name="ps_acc", bufs=1, space="PSUM"))

        cst = const.tile([128, CW], F32, tag="cst")
        nc.sync.dma_start(cst[:, :], cst_d)
        xb = cst[:, COFF["xb"]:COFF["xb"] + D]
        wy = cst[:, COFF["wy"]:COFF["wy"] + D]
        ygb = cst[:, COFF["ygb"]:COFF["ygb"] + H]
        mT = cst[:, COFF["mt"]:COFF["mt"] + H]
        c0t = cst[:, COFF["c0t"]:COFF["c0t"] + NB * C]
        c0n = cst[0:C, COFF["c0n"]:COFF["c0n"] + D]
        ncn0c = cst[0:C, COFF["ncn0"]:COFF["ncn0"] + 1]
        ident = cst[:, COFF["ident"]:COFF["ident"] + 128]
        b1b = cst[0:C, COFF["b1b"]:COFF["b1b"] + D]
        b2b = cst[0:C, COFF["b2b"]:COFF["b2b"] + D]
        w1 = cst[:, COFF["w1"]:COFF["w1"] + NB * D]
        w2 = cst[:, COFF["w2"]:COFF["w2"] + NB * D]
        ones_c = const.tile([128, 1], F16, tag="ones_c")
        nc.gpsimd.memset(ones_c[:, :], 1.0)
        ones_r = const.tile([1, 128], F32, tag="ones_r")
        nc.gpsimd.memset(ones_r[:, :], 1.0)
        ident16 = const.tile([128, 128], F16, tag="ident16")
        nc.vector.tensor_copy(ident16[:, :], ident)
        c0th = const.tile([128, NB * C], F16, tag="c0th")
        nc.vector.tensor_copy(c0th[:, :], c0t)
        c0tl = const.tile([128, NB * C], F16, tag="c0tl")
        nc.vector.tensor_sub(c0tl[:, :], c0t, c0th[:, :])

        def group_labels(g_i, tokTg, cT, ncn_col):
            """tokTg: [128, NB*512] (block b at cols b*512); cT: [128, NB*C];
            ncn_col: [C, 1] = -0.5*||c||^2. Returns 4 one-hot U tiles [128, C]."""
            cTh, cTl = cT
            tokTgh, tokTgl = tokTg
            psGp = ps_g.tile([C, 512], F32, tag="g")
            for b in range(NB):
                nc.tensor.matmul(
                    psGp[:, :], cTh[:, b * C:(b + 1) * C],
                    tokTgh[:, b * 512:(b + 1) * 512],
                    start=(b == 0), stop=False,
                )
                nc.tensor.matmul(
                    psGp[:, :], cTl[:, b * C:(b + 1) * C],
                    tokTgh[:, b * 512:(b + 1) * 512],
                    start=False, stop=False,
                )
                nc.tensor.matmul(
                    psGp[:, :], cTh[:, b * C:(b + 1) * C],
                    tokTgl[:, b * 512:(b + 1) * 512],
                    start=False, stop=(b == NB - 1),
                )
            qg = tmp.tile([C, 512], F32, tag="qg")
            nc.vector.tensor_scalar(qg[:, :], psGp[:, :], ncn_col, None,
                                    op0=OP.add)
            us = []
            for i in range(4):
                t_i = g_i * 4 + i
                psQ = ps_qtr.tile([128, 16], F32, tag="qtr")
                nc.tensor.transpose(psQ[:, 0:C], qg[:, i * 128:(i + 1) * 128],
                                    ident[0:C, 0:C])
                q = small.tile([128, C], F32, tag="q")
                nc.vector.tensor_copy(q[:, :], psQ[:, 0:C])
                mx = small.tile([128, 8], F32, tag="mx")
                nc.vector.max(mx[:, :], q[:, :])
                u = small.tile([128, C], F16, tag="u")
                nc.vector.tensor_scalar(
                    u[:, :], q[:, :], mx[:, 0:1], mT[:, t_i:t_i + 1],
                    op0=OP.is_ge, op1=OP.mult,
                )
                us.append(u)
            return us

        def group_sums(g_i, us, tokNg, psS, psCnt):
            tokNgh, tokNgl = tokNg
            for i in range(4):
                t_i = g_i * 4 + i
                first = (t_i == 0)
                last = (t_i == NTILE - 1)
                nc.tensor.matmul(psS[0][:, :], us[i][:, :],
                                 tokNgh[:, i * D:i * D + D // 2],
                                 start=first, stop=False)
                nc.tensor.matmul(psS[0][:, :], us[i][:, :],
                                 tokNgl[:, i * D:i * D + D // 2],
                                 start=False, stop=False)
                nc.tensor.matmul(psS[1][:, :], us[i][:, :],
                                 tokNgh[:, i * D + D // 2:(i + 1) * D],
                                 start=first, stop=False)
                nc.tensor.matmul(psS[1][:, :], us[i][:, :],
                                 tokNgl[:, i * D + D // 2:(i + 1) * D],
                                 start=False, stop=last)
                nc.tensor.matmul(psCnt[:, 0:1], us[i][:, :], ones_c[:, :],
                                 start=False, stop=last)

        def iter_finish(psS, psCnt, cN_prev):
            """psS: 2x psum [C, D/2]; psCnt: psum [C, 16]; cN_prev: [C, D].
            Returns (cN, cT, ncnb): natural + transposed centroids, -cn2/2."""
            cb = small.tile([C, 1], F32, tag="cb")
            nc.vector.tensor_copy(cb[:, :], psCnt[:, 0:1])
            cmax = small.tile([C, 1], F32, tag="cmax")
            nc.vector.tensor_scalar(cmax[:, :], cb[:, :], 1.0, None, op0=OP.max)
            rcp = small.tile([C, 1], F32, tag="rcp")
            nc.vector.reciprocal(rcp[:, :], cmax[:, :])
            mb = small.tile([C, 1], F32, tag="mb")
            nc.vector.tensor_scalar(mb[:, :], cb[:, :], 0.5, None, op0=OP.is_ge)
            imb = small.tile([C, 1], F32, tag="imb")
            nc.vector.tensor_scalar(imb[:, :], mb[:, :], -1.0, 1.0,
                                    op0=OP.mult, op1=OP.add)
            cN = ctp.tile([C, D], F32, tag="cn")
            for hf in range(2):
                hs = slice(hf * (D // 2), (hf + 1) * (D // 2))
                tmp1 = small.tile([C, D // 2], F32, tag="tmp1")
                nc.vector.tensor_scalar(tmp1[:, :], psS[hf][:, :], rcp[:, :],
                                        mb[:, :], op0=OP.mult, op1=OP.mult)
                nc.vector.scalar_tensor_tensor(
                    cN[:, hs], cN_prev[:, hs], imb[:, :], tmp1[:, :],
                    op0=OP.mult, op1=OP.add)
            sq = tmp.tile([C, D], F32, tag="sq")
            nc.vector.tensor_tensor(sq[:, :], cN[:, :], cN[:, :], op=OP.mult)
            nr = small.tile([C, 1], F32, tag="nr")
            nc.vector.tensor_reduce(nr[:, :], sq[:, :],
                                    axis=mybir.AxisListType.X, op=OP.add)
            nr2 = small.tile([C, 1], F32, tag="nr2")
            nc.vector.tensor_scalar(nr2[:, :], nr[:, :], -0.5, None, op0=OP.mult)
            cT = ctp.tile([128, NB * C], F32, tag="cfin")
            cTh = ctp.tile([128, NB * C], F16, tag="cfh")
            cTl = ctp.tile([128, NB * C], F16, tag="cfl")
            for b in range(NB):
                sl = slice(b * C, (b + 1) * C)
                psT = ps_g.tile([128, 16], F32, tag="g")
                nc.tensor.transpose(psT[:, 0:C], cN[:, b * 128:(b + 1) * 128],
                                    ident[0:C, 0:C])
                nc.vector.tensor_copy(cT[:, sl], psT[:, 0:C])
                nc.vector.tensor_copy(cTh[:, sl], psT[:, 0:C])
                nc.vector.tensor_sub(cTl[:, sl], cT[:, sl], cTh[:, sl])
            return cN, cT, (cTh, cTl), nr2

        # ---------------- pass 0 + fused k-means iter 1 ----------------
        psA = ps_acc.tile([C, 512], F32, tag="acc0")
        psS1 = ps_acc.tile([C, D // 2], F32, tag="acc1")
        psS = [psA[:, 0:D // 2], psS1]
        psCnt = psA[:, D // 2:D // 2 + 16]
        for g_i in range(NTILE // 4):
            tokNgh = io.tile([128, 4 * D], F16, tag="tokh")
            tokNgl = io.tile([128, 4 * D], F16, tag="tokl")
            tokTgh = io.tile([128, NB * 512], F16, tag="tokth")
            tokTgl = io.tile([128, NB * 512], F16, tag="toktl")
            for i in range(4):
                t_i = g_i * 4 + i
                ft = io.tile([128, D], F32, tag="ft")
                nc.sync.dma_start(ft[:, :], feat[t_i * 128:(t_i + 1) * 128, :])
                t1 = tmp.tile([128, D], F32, tag="t1")
                nc.vector.scalar_tensor_tensor(
                    t1[:, :], wy, ygb[:, t_i:t_i + 1], ft[:, :],
                    op0=OP.mult, op1=OP.add,
                )
                t2 = tmp.tile([128, D], F32, tag="t2")
                nc.gpsimd.tensor_tensor(t2[:, :], t1[:, :], xb, op=OP.add)
                tok = tmp.tile([128, D], F32, tag="tok")
                nc.scalar.mul(tok[:, :], t2[:, :], mT[:, t_i:t_i + 1])
                nsl = slice(i * D, (i + 1) * D)
                nc.scalar.copy(tokNgh[:, nsl], tok[:, :])
                nc.vector.tensor_sub(tokNgl[:, nsl], tok[:, :], tokNgh[:, nsl])
                ptr = ps_tr.tile([128, 2 * D], F16, tag="tr")
                for b in range(NB):
                    nc.tensor.transpose(
                        ptr[:, b * 128:(b + 1) * 128],
                        tokNgh[:, i * D + b * 128:i * D + (b + 1) * 128],
                        ident16)
                    nc.tensor.transpose(
                        ptr[:, D + b * 128:D + (b + 1) * 128],
                        tokNgl[:, i * D + b * 128:i * D + (b + 1) * 128],
                        ident16)
                for b in range(NB):
                    dsth = tokTgh[:, b * 512 + i * 128:b * 512 + (i + 1) * 128]
                    dstl = tokTgl[:, b * 512 + i * 128:b * 512 + (i + 1) * 128]
                    if b % 2 == 0:
                        nc.vector.tensor_copy(dsth, ptr[:, b * 128:(b + 1) * 128])
                        nc.scalar.copy(dstl, ptr[:, D + b * 128:D + (b + 1) * 128])
                    else:
                        nc.scalar.copy(dsth, ptr[:, b * 128:(b + 1) * 128])
                        nc.vector.tensor_copy(dstl, ptr[:, D + b * 128:D + (b + 1) * 128])
            nc.sync.dma_start(
                tokNh[g_i * 512:(g_i + 1) * 512, :]
                    .rearrange("(i p) e -> p i e", i=4),
                tokNgh[:, :].rearrange("p (i e) -> p i e", i=4))
            nc.sync.dma_start(
                tokNl[g_i * 512:(g_i + 1) * 512, :]
                    .rearrange("(i p) e -> p i e", i=4),
                tokNgl[:, :].rearrange("p (i e) -> p i e", i=4))
            nc.sync.dma_start(
                tokTh3[:, :, g_i * 512:(g_i + 1) * 512],
                tokTgh[:, :].rearrange("p (b t) -> p b t", b=NB))
            nc.sync.dma_start(
                tokTl3[:, :, g_i * 512:(g_i + 1) * 512],
                tokTgl[:, :].rearrange("p (b t) -> p b t", b=NB))
            us = group_labels(g_i, (tokTgh, tokTgl), (c0th, c0tl), ncn0c)
            group_sums(g_i, us, (tokNgh, tokNgl), psS, psCnt)
        cN, cT, cT16, ncn = iter_finish(psS, psCnt, c0n)

        # ---------------- k-means iterations 2..ITERS ----------------
        for it in range(1, ITERS):
            psA = ps_acc.tile([C, 512], F32, tag="acc0")
            psS1 = ps_acc.tile([C, D // 2], F32, tag="acc1")
            psS = [psA[:, 0:D // 2], psS1]
            psCnt = psA[:, D // 2:D // 2 + 16]
            for g_i in range(NTILE // 4):
                tokNgh = io.tile([128, 4 * D], F16, tag="tokh")
                nc.sync.dma_start(
                    tokNgh[:, :].rearrange("p (i e) -> p i e", i=4),
                    tokNh[g_i * 512:(g_i + 1) * 512, :]
                        .rearrange("(i p) e -> p i e", i=4))
                tokNgl = io.tile([128, 4 * D], F16, tag="tokl")
                nc.sync.dma_start(
                    tokNgl[:, :].rearrange("p (i e) -> p i e", i=4),
                    tokNl[g_i * 512:(g_i + 1) * 512, :]
                        .rearrange("(i p) e -> p i e", i=4))
                tokTgh = io.tile([128, NB * 512], F16, tag="tokth")
                nc.sync.dma_start(
                    tokTgh[:, :].rearrange("p (b t) -> p b t", b=NB),
                    tokTh3[:, :, g_i * 512:(g_i + 1) * 512])
                tokTgl = io.tile([128, NB * 512], F16, tag="toktl")
                nc.sync.dma_start(
                    tokTgl[:, :].rearrange("p (b t) -> p b t", b=NB),
                    tokTl3[:, :, g_i * 512:(g_i + 1) * 512])
                us = group_labels(g_i, (tokTgh, tokTgl), cT16, ncn)
                group_sums(g_i, us, (tokNgh, tokNgl), psS, psCnt)
            cN, cT, cT16, ncn = iter_finish(psS, psCnt, cN)

        # ---------------- projection MLP ----------------
        h1 = tmp.tile([C, D], F32, tag="h1")
        for half in range(2):
            hs = slice(half * (D // 2), (half + 1) * (D // 2))
            psH = ps_g.tile([C, D // 2], F32, tag="g")
            for b in range(NB):
                nc.tensor.matmul(
                    psH[:, :], cT[:, b * C:(b + 1) * C],
                    w1[:, b * D + half * (D // 2): b * D + (half + 1) * (D // 2)],
                    start=(b == 0), stop=(b == NB - 1),
                )
            hb = tmp.tile([C, D // 2], F32, tag="hb")
            nc.vector.tensor_tensor(hb[:, :], psH[:, :], b1b[:, hs], op=OP.add)
            nc.scalar.activation(h1[:, hs], hb[:, :], _GELU)
        h1t = tmp.tile([128, NB * C], F32, tag="h1t")
        for b in range(NB):
            psT = ps_g.tile([128, 16], F32, tag="g")
            nc.tensor.transpose(psT[:, 0:C], h1[:, b * 128:(b + 1) * 128],
                                ident[0:C, 0:C])
            nc.vector.tensor_copy(h1t[:, b * C:(b + 1) * C], psT[:, 0:C])
        osb = tmp.tile([C, D], F32, tag="osb")
        for half in range(2):
            hs = slice(half * (D // 2), (half + 1) * (D // 2))
            psO = ps_g.tile([C, D // 2], F32, tag="g")
            for b in range(NB):
                nc.tensor.matmul(
                    psO[:, :], h1t[:, b * C:(b + 1) * C],
                    w2[:, b * D + half * (D // 2): b * D + (half + 1) * (D // 2)],
                    start=(b == 0), stop=(b == NB - 1),
                )
            nc.vector.tensor_tensor(osb[:, hs], psO[:, :], b2b[:, hs], op=OP.add)
        nc.sync.dma_start(out_d, osb[:, :])

    nc.compile()
    return nc


def feat_pair_src(tokN, t_i):
    return (tokN[t_i * 128:(t_i + 2) * 128, :]
            .rearrange("(i p) e -> p i e", i=2))


def _host_prep(feat, mask, boxes, Wp, bp, W1, b1, W2, b2, init_idx):
    """Per-core input dicts. All small O(K*(H+W)*D) prep; feat is viewed."""
    feat = np.ascontiguousarray(np.asarray(feat, dtype=np.float32))
    mask = np.asarray(mask, dtype=np.float32)
    boxes = np.asarray(boxes, dtype=np.float32)
    Wp = np.asarray(Wp, dtype=np.float32)
    bp = np.asarray(bp, dtype=np.float32)
    W1 = np.asarray(W1, dtype=np.float32)
    b1 = np.asarray(b1, dtype=np.float32)
    W2 = np.asarray(W2, dtype=np.float32)
    b2 = np.asarray(b2, dtype=np.float32)
    init_idx = np.asarray(init_idx)

    w1sb = np.ascontiguousarray(
        W1.reshape(NB, 128, D).transpose(1, 0, 2).reshape(128, NB * D))
    w2sb = np.ascontiguousarray(
        W2.reshape(NB, 128, D).transpose(1, 0, 2).reshape(128, NB * D))
    b1b = np.ascontiguousarray(np.broadcast_to(b1, (C, D)))
    b2b = np.ascontiguousarray(np.broadcast_to(b2, (C, D)))
    ident = np.eye(128, dtype=np.float32)

    maps = []
    for k in range(K):
        top, left, bot, right = boxes[k]
        xg = np.arange(W, dtype=np.float32) / np.float32(W) * (right - left) + left
        xg = np.clip(xg / np.float32(RAW_W - 1), 0.0, 1.0).astype(np.float32)
        yg = np.arange(H, dtype=np.float32) / np.float32(H) * (bot - top) + top
        yg = np.clip(yg / np.float32(RAW_H - 1), 0.0, 1.0).astype(np.float32)

        xb = (xg[:, None] * Wp[0][None, :] + bp[None, :]).astype(np.float32)
        wy = np.ascontiguousarray(np.broadcast_to(Wp[1], (128, D)))
        ygb = np.ascontiguousarray(np.broadcast_to(yg[None, :], (128, H)))
        mk = (mask[k] > 0).astype(np.float32)
        maskT = np.ascontiguousarray(mk.T)

        idx = init_idx[k].astype(np.int64)
        hr, wr = idx // W, idx % W
        fr = feat[k].reshape(NT, D)[idx]
        t1 = fr + yg[hr][:, None] * Wp[1][None, :]
        c0 = (t1 + xb[wr]).astype(np.float32)  # mask at init_idx is 1 by setup
        c0t = np.ascontiguousarray(
            c0.T.reshape(NB, 128, C).transpose(1, 0, 2).reshape(128, NB * C))
        ncn0c = (-0.5 * np.sum(c0.astype(np.float32) ** 2,
                                axis=1)).astype(np.float32)[:, None]

        cstbuf = np.zeros((128, CW), dtype=np.float32)
        cstbuf[:, COFF["xb"]:COFF["xb"] + D] = xb
        cstbuf[:, COFF["wy"]:COFF["wy"] + D] = wy
        cstbuf[:, COFF["ygb"]:COFF["ygb"] + H] = ygb
        cstbuf[:, COFF["mt"]:COFF["mt"] + H] = maskT
        cstbuf[:, COFF["c0t"]:COFF["c0t"] + NB * C] = c0t
        cstbuf[0:C, COFF["c0n"]:COFF["c0n"] + D] = c0
        cstbuf[0:C, COFF["ncn0"]:COFF["ncn0"] + 1] = ncn0c
        cstbuf[:, COFF["ident"]:COFF["ident"] + 128] = ident
        cstbuf[0:C, COFF["b1b"]:COFF["b1b"] + D] = b1b
        cstbuf[0:C, COFF["b2b"]:COFF["b2b"] + D] = b2b
        cstbuf[:, COFF["w1"]:COFF["w1"] + NB * D] = w1sb
        cstbuf[:, COFF["w2"]:COFF["w2"] + NB * D] = w2sb
        maps.append({
            "feat": feat[k].reshape(NT, D),
            "consts": cstbuf,
        })
    return maps


def run(trace=False, **inputs):
    if "nc" not in _CACHE:
        _CACHE["nc"] = _build_program()
    nc = _CACHE["nc"]
    in_maps = _host_prep(
        inputs["feat"], inputs["mask"], inputs["boxes"], inputs["Wp"],
        inputs["bp"], inputs["W1"], inputs["b1"], inputs["W2"], inputs["b2"],
        inputs["init_idx"])
    res = run_bass_kernel_spmd(nc, in_maps, core_ids=list(range(K)),
                               trace=trace)
    out = np.stack([np.asarray(res.results[k]["out"]) for k in range(K)])
    return out.astype(np.float32), res


def kernel(**inputs):
    out, _ = run(trace=False, **inputs)
    return out
